# revision 1
# baseline (speedup 1.0000x reference)
"""Trainium2 Bass kernel for nn_Attention_54142357733562 (linear/sparse attention).

Reference math (per batch b, with x flattened to [C, N]):
    Q = wq @ x ; K = wk @ x ; V = wv @ x            (1x1 convs, + zero biases)
    Qn = Q / ||Q||_c ; Kn = K / ||K||_c             (L2 norm over channel dim)
    k_sum = sum_n Kn + EPS                          [Cqk]
    tailor = 1 / (N + Qn^T k_sum)                   [N]
    kv = Kn V^T                                     [Cqk, C]
    out = (value_sum + kv^T Qn) * tailor            [C, N]

Algebraic reformulation used here (avoids materializing Qn / tailor):
    s[n]   = ||Q[:, n]||
    den[n] = N*s[n] + Q[:, n]. k_sum
    out[c,n] = (U[c,n] + value_sum[c]*s[n]) / den[n],   U = kv^T Q
which is computed as a single matmul with the scale folded into the rhs:
    Q''[m,n] = [Q; s][m,n] / den[n]      (per-n scale, applied in [n,m] layout)
    out[c,n] = sum_m [kv; value_sum][m,c] * Q''[m,n]

Sharding: 8 cores = 4 batches x 2 N-halves. Phase 1 computes per-shard
partial (kv | k_sum | value_sum) = [Kn|1]^T [V|1]; an AllReduce over the
2-core pair completes the N reduction; phase 2 computes outputs for the
shard's N range. x is read once and out written once per core (67 MB).
"""

import os
import numpy as np
from contextlib import ExitStack

import concourse.bass as bass
import concourse.mybir as mybir
import concourse.tile as tile
from concourse import bacc
from concourse.bass_utils import run_bass_kernel_spmd
from concourse.masks import make_identity

F32 = mybir.dt.float32
F32R = mybir.dt.float32r

C = 256
CQK = 32
J = 2 * CQK + C  # 320 = stacked [Q|K|V] output channels
EPS = 1e-6
P = 128
NT = 512  # macro-tile width along N
ST = NT // P  # 4 sub-tiles per macro


def emit_attention(tc, xs, wt, out, nsh, n_total, groups, mm_dtype="f32r",
                   use_collective=True, phases=(1, 2)):
    """Emit the per-core SPMD program.

    xs : DRAM [C, nsh]  per-core shard of x (C-major)
    wt : DRAM [C, J]    stacked transposed weights [wq.T | wk.T | wv.T]
    out: DRAM [C, nsh]  per-core shard of the output
    """
    nc = tc.nc
    NM = nsh // NT
    SROW = nsh // P

    MDT = F32R if mm_dtype == "f32r" else F32

    xs_r = xs.rearrange("(o p) n -> p o n", p=P)  # [128, 2, nsh]
    out_r = out.rearrange("(o p) n -> p o n", p=P)
    wt_r = wt.rearrange("(o p) j -> p o j", p=P)  # [128, 2, 320]

    mult = mybir.AluOpType.mult

    with ExitStack() as ctx:
        singles = ctx.enter_context(tc.tile_pool(name="singles", bufs=1))
        dram = ctx.enter_context(tc.tile_pool(name="dram", bufs=1, space="DRAM"))

        wsb = singles.tile([P, 2, J], MDT)
        nc.sync.dma_start(wsb, wt_r)
        ident = singles.tile([P, P], F32)
        make_identity(nc, ident)
        ones_r = singles.tile([P, 1], MDT)
        ones_f = singles.tile([P, 1], F32)
        nc.vector.memset(ones_f, 1.0)
        nc.vector.tensor_copy(ones_r, ones_f)
        ident_r = singles.tile([P, P], MDT)
        nc.vector.tensor_copy(ident_r, ident)
        # stacked identity [I33; 0; I33; 0] for merging kv column groups
        merge_f = singles.tile([P, CQK + 1], F32)
        nc.gpsimd.memset(merge_f, 0.0)
        nc.gpsimd.affine_select(
            out=merge_f, in_=merge_f, compare_op=mybir.AluOpType.not_equal,
            fill=1.0, base=0, pattern=[[-1, CQK + 1]], channel_multiplier=1)
        nc.gpsimd.affine_select(
            out=merge_f, in_=merge_f, compare_op=mybir.AluOpType.not_equal,
            fill=1.0, base=-64, pattern=[[-1, CQK + 1]], channel_multiplier=1)
        merge_i = singles.tile([P, CQK + 1], MDT)
        nc.vector.tensor_copy(merge_i, merge_f)

        # stash row layout (W=66): [Q 0:32 | s 32 | ||K|| 33 | K 34:66]
        # ([Q|s] contiguous at 0:33 is what phase 2 consumes)
        SW = 2 * CQK + 2
        stash = singles.tile([P, SROW, SW], F32)

        # ---------------- phase 1: QKV + partial [Kn|1]^T [V|1] ----------------
        KW = (CQK + 1) + (C + 2)  # 33 + 258 = 291 (f32r needs even moving dim)
        with ExitStack() as p1:
            xp = p1.enter_context(tc.tile_pool(name="xp", bufs=6))
            kvb = p1.enter_context(tc.tile_pool(name="kvb", bufs=6))
            scr = p1.enter_context(tc.tile_pool(name="scr", bufs=6))
            ps_qkv = p1.enter_context(tc.tile_pool(name="ps_qkv", bufs=3, space="PSUM"))
            ps_kv = p1.enter_context(tc.tile_pool(name="ps_kv", bufs=1, space="PSUM"))

            kv_acc = ps_kv.tile([P, 1, 512], F32)  # single accumulator bank

            HS = 2  # sub-tiles per half-macro (psum tile = 2 banks, bufs=3)
            for m in range(NM):
                xt = xp.tile([P, 2, NT], MDT)
                for o in range(2):
                    nc.sync.dma_start(xt[:, o, :],
                                      xs_r[:, o, m * NT:(m + 1) * NT])

                kvt = kvb.tile([P, ST, KW], MDT)
                # ones columns once per macro (GPSIMD, SBUF-only)
                nc.gpsimd.tensor_copy(
                    kvt[:, :, CQK:CQK + 1],
                    ones_r[:, None, :].to_broadcast((P, ST, 1)))
                nc.gpsimd.tensor_copy(
                    kvt[:, :, KW - 2:KW],
                    ones_r[:, None, :].to_broadcast((P, ST, 2)))

                mst = stash[:, m * ST:(m + 1) * ST, :]  # [128, 4, 66]
                for h in range(ST // HS):
                    ps = ps_qkv.tile([P, HS, 512], F32)  # 2 banks
                    for s2 in range(HS):
                        s = h * HS + s2
                        for o in range(2):
                            nc.tensor.matmul(
                                ps[:, s2, 0:J],
                                xt[:, o, s * P:(s + 1) * P],
                                wsb[:, o, :],
                                start=(o == 0),
                                stop=(o == 1),
                            )

                    r0 = m * ST + h * HS
                    st_sl = stash[:, r0:r0 + HS, :]  # [128, 2, 66]
                    kv_sl = kvt[:, h * HS:h * HS + HS, :]

                    # PSUM -> SBUF: Q,K into stash (one strided copy); V into
                    # kvt (alternating engine)
                    qk_dst = bass.AP(
                        tensor=st_sl.tensor,
                        offset=st_sl.offset,
                        ap=[st_sl.ap[0], st_sl.ap[1],
                            [CQK + 2, 2], [1, CQK]],
                    )
                    nc.vector.tensor_copy(
                        qk_dst,
                        ps[:, :, 0:2 * CQK].rearrange(
                            "p h (g c) -> p h g c", g=2))
                    if (2 * m + h) % 4 == 2:
                        nc.vector.tensor_copy(kv_sl[:, :, CQK + 1:CQK + 1 + C],
                                              ps[:, :, 2 * CQK:J])
                    else:
                        nc.scalar.copy(kv_sl[:, :, CQK + 1:CQK + 1 + C],
                                       ps[:, :, 2 * CQK:J])

                # per-macro normalization chain (batched over all 4 sub-tiles)
                qk_view = bass.AP(
                    tensor=mst.tensor,
                    offset=mst.offset,
                    ap=[mst.ap[0], mst.ap[1], [CQK + 2, 2], [1, CQK]],
                )
                sq = scr.tile([P, ST, 2, CQK], F32, tag="sq")
                nc.gpsimd.tensor_tensor(sq, qk_view, qk_view, mult)
                ssq = scr.tile([P, ST, 2], F32, tag="ssq")
                nc.vector.reduce_sum(ssq, sq, axis=mybir.AxisListType.X)
                # sqrt -> stash cols 32 (s) and 33 (||K||)
                nc.scalar.sqrt(mst[:, :, CQK:CQK + 2], ssq)
                rkn = scr.tile([P, ST, 1], F32, tag="rkn")
                nc.vector.reciprocal(rkn, mst[:, :, CQK + 1:CQK + 2])
                # Kn = K / ||K|| (GPSIMD, SBUF only)
                nc.gpsimd.tensor_tensor(kvt[:, :, 0:CQK],
                                        mst[:, :, CQK + 2:SW],
                                        rkn.to_broadcast((P, ST, CQK)), mult)

                # accumulate [Kn|1]^T [V|1|1] -> [33, 258] (two accumulators
                # so consecutive macros' matmuls have no mutual PE ordering)
                for s in range(ST):
                    nc.tensor.matmul(
                        kv_acc[0:CQK + 1, 0, 0:C + 2],
                        kvt[:, s, 0:CQK + 1],
                        kvt[:, s, CQK + 1:KW],
                        start=(m == 0 and s == 0),
                        stop=(m == NM - 1 and s == ST - 1),
                    )

            kv_sb = singles.tile([CQK + 1, C + 2], F32)
            nc.vector.tensor_copy(kv_sb, kv_acc[0:CQK + 1, 0, 0:C + 2])

        cc_in = dram.tile([CQK + 1, C + 2], F32)
        cc_out = dram.tile([CQK + 1, C + 2], F32)
        nc.sync.dma_start(cc_in, kv_sb)
        if use_collective:
            nc.gpsimd.collective_compute(
                "AllReduce",
                mybir.AluOpType.add,
                replica_groups=groups,
                ins=[cc_in.opt()],
                outs=[cc_out.opt()],
            )
        else:
            nc.sync.dma_start(cc_out, cc_in)

        # kvp[m, c]: rows 0:32 = kv, row 32 = value_sum
        kvp_f32 = singles.tile([CQK + 1, C], F32)
        nc.sync.dma_start(kvp_f32, cc_out[:, 0:C])
        kvp = singles.tile([CQK + 1, C], MDT)
        nc.vector.tensor_copy(kvp, kvp_f32)
        # ksum[p, 0:32] = k_sum + EPS (broadcast over partitions), col 32 = N
        ksum = singles.tile([P, CQK + 1], F32)
        nc.sync.dma_start(ksum[:, 0:CQK],
                          cc_out[0:CQK, C:C + 1].partition_broadcast(P))
        nc.vector.tensor_scalar_add(ksum[:, 0:CQK], ksum[:, 0:CQK], EPS)
        nc.vector.memset(ksum[:, CQK:CQK + 1], float(n_total))

        if 2 not in phases:
            # debug/measurement mode: write something so 'out' has a writer
            nc.sync.dma_start(out_r[:, :, 0:NT],
                              xs_r[:, :, 0:NT].bitcast(F32))
            return
        # ---------------- phase 2: out = [kv|vs]^T ([Q;s]/den) ----------------
        with ExitStack() as p2:
            scr2 = ctx.enter_context(tc.tile_pool(name="scr2", bufs=3))
            qtp = ctx.enter_context(tc.tile_pool(name="qtp", bufs=3))
            outp = ctx.enter_context(tc.tile_pool(name="outp", bufs=4))
            ps_qt = p2.enter_context(tc.tile_pool(name="ps_qt", bufs=1, space="PSUM"))
            ps_out = p2.enter_context(tc.tile_pool(name="ps_out", bufs=6, space="PSUM"))

            MP = 2 * ST  # process macro PAIRS: [128, 8, 33] batches
            for mp in range(NM // 2):
                st_sl = stash[:, mp * MP:(mp + 1) * MP, 0:CQK + 1]  # [128,8,33]

                prod = scr2.tile([P, MP, CQK + 1], F32, tag="prod")
                nc.gpsimd.tensor_tensor(
                    prod, st_sl,
                    ksum[:, None, :].to_broadcast((P, MP, CQK + 1)), mult)
                den = scr2.tile([P, MP, 1], F32, tag="den")
                nc.vector.reduce_sum(den, prod, axis=mybir.AxisListType.X)
                d = scr2.tile([P, MP, 1], F32, tag="d")
                nc.vector.reciprocal(d, den)
                # qsc = [Q; s] * d[n] (f32r so the transposes run at 1.5 cyc)
                qsc = scr2.tile([P, MP, CQK + 1], MDT, tag="qsc")
                nc.gpsimd.tensor_tensor(qsc, st_sl,
                                        d.to_broadcast((P, MP, CQK + 1)), mult)

                qt_ps = ps_qt.tile([CQK + 1, MP, P], MDT)  # [33, 8, 128] 2 banks
                for s in range(MP):
                    nc.tensor.transpose(qt_ps[:, s, :], qsc[:, s, :], ident_r)
                qt_sb = qtp.tile([CQK + 1, MP * P], MDT)
                # per-half drains on different engines: each macro-half's
                # final matmuls wait only on their own half of qt
                nc.vector.tensor_copy(qt_sb[:, 0:MP * P // 2],
                                      qt_ps[:, 0:MP // 2, :])
                nc.scalar.copy(qt_sb[:, MP * P // 2:MP * P],
                               qt_ps[:, MP // 2:MP, :])

                for mh in range(2):
                    m = mp * 2 + mh
                    ot = outp.tile([P, 2, NT], F32)
                    # fully independent per-block matmul->copy->DMA chains,
                    # each on its own 1-bank psum tile (6-way rotation)
                    for blk in range(2):
                        o_ps = ps_out.tile([P, NT], F32, tag="o_ps")
                        nc.tensor.matmul(
                            o_ps,
                            kvp[:, blk * P:(blk + 1) * P],
                            qt_sb[:, mh * NT:(mh + 1) * NT],
                            start=True,
                            stop=True,
                        )
                        if blk == 0:
                            nc.vector.tensor_copy(ot[:, blk, :], o_ps)
                        else:
                            nc.scalar.copy(ot[:, blk, :], o_ps)
                        nc.sync.dma_start(out_r[:, blk, m * NT:(m + 1) * NT],
                                          ot[:, blk, :])


def build_attention_nc(nsh, n_total, num_cores, groups, mm_dtype="f32r",
                       repeat=1, use_collective=True, phases=(1, 2)):
    nc = bacc.Bacc("TRN2", target_bir_lowering=False, debug=False,
                   num_devices=num_cores)
    MDT = F32R if mm_dtype == "f32r" else F32
    xs = nc.dram_tensor("xs", [C, nsh], MDT, kind="ExternalInput").ap()
    wt = nc.dram_tensor("wt", [C, J], MDT, kind="ExternalInput").ap()
    out = nc.dram_tensor("out", [C, nsh], F32, kind="ExternalOutput").ap()
    with tile.TileContext(nc) as tc:
        for _ in range(repeat):
            emit_attention(tc, xs, wt, out, nsh, n_total, groups, mm_dtype,
                           use_collective=use_collective, phases=phases)
    nc.compile()
    return nc


_NC_CACHE = {}


def _get_nc(nsh, n_total, num_cores, groups_key, mm_dtype="f32r"):
    key = (nsh, n_total, num_cores, groups_key, mm_dtype)
    if key not in _NC_CACHE:
        groups = [list(g) for g in groups_key]
        _NC_CACHE[key] = build_attention_nc(nsh, n_total, num_cores, groups,
                                            mm_dtype)
    return _NC_CACHE[key]


def _kernel_numpy(x, wq, bq, wk, bk, wv, bv):
    """Plain numpy fallback (used only for nonzero biases)."""
    b, c, h, w = x.shape
    n = h * w
    xf = x.reshape(b, c, n).astype(np.float64)
    Q = np.einsum("oc,bcn->bon", wq.astype(np.float64), xf) + bq.astype(np.float64)[None, :, None]
    K = np.einsum("oc,bcn->bon", wk.astype(np.float64), xf) + bk.astype(np.float64)[None, :, None]
    V = np.einsum("oc,bcn->bon", wv.astype(np.float64), xf) + bv.astype(np.float64)[None, :, None]
    Qn = Q / np.linalg.norm(Q, axis=1, keepdims=True)
    Kn = K / np.linalg.norm(K, axis=1, keepdims=True)
    k_sum = Kn.sum(-1) + EPS
    tailor = 1.0 / (n + np.einsum("bmn,bm->bn", Qn, k_sum))
    value_sum = V.sum(-1)
    kv = np.einsum("bmn,bcn->bmc", Kn, V)
    ms = value_sum[:, :, None] + np.einsum("bmn,bmc->bcn", Qn, kv)
    return (ms * tailor[:, None, :]).reshape(b, c, h, w).astype(np.float32)


def kernel(x, wq, bq, wk, bk, wv, bv):
    x = np.asarray(x, dtype=np.float32)
    B, Cc, H, W = x.shape
    if (any(np.any(np.asarray(b_) != 0) for b_ in (bq, bk, bv))
            or Cc != C or wq.shape != (CQK, C) or wv.shape != (C, C)
            or (H * W) % (2 * NT) != 0 or B != 4):
        return _kernel_numpy(x, wq, bq, wk, bk, wv, bv)
    N = H * W
    ncores = 8
    shards_per_batch = ncores // B  # 2
    nsh = N // shards_per_batch  # 32768
    groups_key = tuple(
        tuple(range(b * shards_per_batch, (b + 1) * shards_per_batch))
        for b in range(B)
    )

    wt = np.ascontiguousarray(
        np.concatenate([np.asarray(wq).T, np.asarray(wk).T, np.asarray(wv).T],
                       axis=1).astype(np.float32))

    nc = _get_nc(nsh, N, ncores, groups_key)

    xr = x.reshape(B, Cc, N)
    in_maps = []
    for core in range(ncores):
        b, hh = core // shards_per_batch, core % shards_per_batch
        in_maps.append({
            "xs": np.ascontiguousarray(xr[b, :, hh * nsh:(hh + 1) * nsh]),
            "wt": wt,
        })

    res = run_bass_kernel_spmd(nc, in_maps, list(range(ncores)))

    out = np.empty((B, Cc, N), np.float32)
    for core in range(ncores):
        b, hh = core // shards_per_batch, core % shards_per_batch
        out[b, :, hh * nsh:(hh + 1) * nsh] = res.results[core]["out"]
    return out.reshape(B, Cc, H, W)



# revision 10
# speedup vs baseline: 3.2779x; 3.2779x over previous
"""Trainium2 Bass kernel for nn_Attention_54142357733562 (linear attention).

Reference math (per batch b, x flattened to [C, N]):
    Q = wq @ x ; K = wk @ x ; V = wv @ x
    Qn = Q / ||Q||_c ; Kn = K / ||K||_c
    k_sum = sum_n Kn + EPS
    out = (value_sum + kv^T Qn) / (N + Qn^T k_sum),  kv = Kn V^T

Algebraic restructure used here (all matmul inputs bf16):
    s[n] = ||Q[:, n]||;  den[n] = N*s[n] + Q[:, n].k_sum
    G'   = [Kn | 1]^T [x^T | 1]        # [33, 258]: G = Kn x^T, ksum, x_sum, n
    (AllReduce G' over the 2-core pair)
    kvp  = G'[:, 0:256] @ wv^T          # rows 0:32 = kv, row 32 = value_sum
    out[c, n] = sum_m [kvp][m, c] * ([Q; s][m, n] / den[n])
V is never materialized: kv == (Kn x^T) wv^T and value_sum == wv x_sum,
so phase 1 only computes Q, K (64 channels) plus a PE transpose of x.

Sharding: 8 cores = 4 batches x 2 N-halves. x is uploaded in bf16 (16 MiB
per core), out is written in bf16 and upcast on the host; HBM traffic is
half of the f32 version. The AllReduce payload is [33, 258] f32.
"""

import numpy as np
from contextlib import ExitStack

import concourse.bass as bass
import concourse.mybir as mybir
import concourse.tile as tile
from concourse import bacc
from concourse.bass_utils import run_bass_kernel_spmd
from concourse.masks import make_identity

F32 = mybir.dt.float32
BF16 = mybir.dt.bfloat16

C = 256
CQK = 32
P = 128
NT = 1024  # macro-tile width along N
ST = NT // P  # 8
SW = 66    # stash row: [Q 0:32 | s 32 | ||K|| 33 | K 34:66]
GW = 291   # g_in row: [Kn 0:32 | 1 | x^T 33:289 | 1 | 1]
EPS = 1e-6


def emit_attention(tc, xs, wqk, wvt, out, nsh, n_total, groups,
                   use_collective=True, phases=(1, 2)):
    nc = tc.nc
    NM = nsh // NT
    SROW = nsh // P

    xs_r = xs.rearrange("(o p) n -> p o n", p=P)    # [128, 2, nsh]
    out_r = out.rearrange("(o p) n -> p o n", p=P)
    wqk_r = wqk.rearrange("(o p) j -> p o j", p=P)  # [128, 2, 64]
    wvt_r = wvt.rearrange("(o p) c -> p o c", p=P)  # [128, 2, 256]

    mult = mybir.AluOpType.mult

    with ExitStack() as ctx:
        singles = ctx.enter_context(tc.tile_pool(name="singles", bufs=1))
        dram = ctx.enter_context(tc.tile_pool(name="dram", bufs=1, space="DRAM"))

        wqk_sb = singles.tile([P, 2, 2 * CQK], BF16)
        nc.sync.dma_start(wqk_sb, wqk_r)
        wvt_sb = singles.tile([P, 2, C], BF16)
        nc.sync.dma_start(wvt_sb, wvt_r)
        ident_f = singles.tile([P, P], F32)
        make_identity(nc, ident_f)
        ident_b = singles.tile([P, P], BF16)
        nc.vector.tensor_copy(ident_b, ident_f)

        stash = singles.tile([P, SROW, SW], BF16)

        # ---------------- phase 1: QK + x^T + G accumulation ----------------
        if 1 in phases:
            with ExitStack() as p1:
                xp = p1.enter_context(tc.tile_pool(name="xp", bufs=4))
                gi = p1.enter_context(tc.tile_pool(name="gi", bufs=3))
                scr = p1.enter_context(tc.tile_pool(name="scr", bufs=3))
                ps_qk = p1.enter_context(
                    tc.tile_pool(name="ps_qk", bufs=2, space="PSUM"))
                ps_xt = p1.enter_context(
                    tc.tile_pool(name="ps_xt", bufs=4, space="PSUM"))
                ps_g = p1.enter_context(
                    tc.tile_pool(name="ps_g", bufs=1, space="PSUM"))

                g_acc = ps_g.tile([P, 512], F32)  # rows 0:33, cols 0:258 used

                xdrain = 0
                for m in range(NM):
                    xt = xp.tile([P, 2, NT], BF16)
                    nc.sync.dma_start(xt, xs_r[:, :, m * NT:(m + 1) * NT])

                    gi_t = gi.tile([P, ST, GW], BF16)
                    nc.gpsimd.memset(gi_t[:, :, CQK:CQK + 1], 1.0)
                    nc.gpsimd.memset(gi_t[:, :, GW - 2:GW], 1.0)

                    qk_ps = ps_qk.tile([P, ST, 64], F32)  # 1 bank
                    for h in range(ST // 2):
                        xt_ps = ps_xt.tile([P, 2, 512], BF16)  # 1 bank
                        for s2 in range(2):
                            s = 2 * h + s2
                            ch = slice(s * P, (s + 1) * P)
                            for o in range(2):
                                nc.tensor.matmul(
                                    qk_ps[:, s, :],
                                    xt[:, o, ch],
                                    wqk_sb[:, o, :],
                                    start=(o == 0),
                                    stop=(o == 1),
                                )
                            for o in range(2):
                                nc.tensor.transpose(
                                    xt_ps[:, s2, o * P:(o + 1) * P],
                                    xt[:, o, ch],
                                    ident_b,
                                )
                        # x^T drain (bf16 -> bf16; 2x on DVE)
                        xt_dst = gi_t[:, 2 * h:2 * h + 2, CQK + 1:CQK + 1 + C]
                        if xdrain % 8 < 5:
                            nc.vector.tensor_copy(xt_dst, xt_ps[:, :, 0:C])
                        else:
                            nc.scalar.copy(xt_dst, xt_ps[:, :, 0:C])
                        xdrain += 1

                    # QK drain once per macro (strided into stash) on ACT
                    mst = stash[:, m * ST:(m + 1) * ST, :]
                    qk_dst = bass.AP(
                        tensor=mst.tensor,
                        offset=mst.offset,
                        ap=[mst.ap[0], mst.ap[1], [CQK + 2, 2], [1, CQK]],
                    )
                    nc.scalar.copy(qk_dst, qk_ps.rearrange("p h (g c) -> p h g c", g=2))

                    # norms: s = ||Q||, ||K||; Kn = K / ||K||
                    qk_view = bass.AP(
                        tensor=mst.tensor,
                        offset=mst.offset,
                        ap=[mst.ap[0], mst.ap[1], [CQK + 2, 2], [1, CQK]],
                    )
                    sq = scr.tile([P, ST, 2, CQK], BF16, tag="sq")
                    nc.vector.tensor_tensor(sq, qk_view, qk_view, mult)
                    ssq = scr.tile([P, ST, 2], BF16, tag="ssq")
                    with nc.allow_low_precision(reason="bf16 norm sums"):
                        nc.vector.reduce_sum(ssq, sq, axis=mybir.AxisListType.X)
                    nc.scalar.sqrt(mst[:, :, CQK:CQK + 2], ssq)
                    rk = scr.tile([P, ST, 1], F32, tag="rk")
                    nc.vector.reciprocal(rk, mst[:, :, CQK + 1:CQK + 2])
                    nc.gpsimd.tensor_tensor(gi_t[:, :, 0:CQK],
                                            mst[:, :, CQK + 2:SW],
                                            rk.to_broadcast((P, ST, CQK)), mult)

                    for s in range(ST):
                        nc.tensor.matmul(
                            g_acc[0:CQK + 1, 0:C + 2],
                            gi_t[:, s, 0:CQK + 1],
                            gi_t[:, s, CQK + 1:GW],
                            start=(m == 0 and s == 0),
                            stop=(m == NM - 1 and s == ST - 1),
                        )

                g_sb = singles.tile([CQK + 1, C + 2], F32)
                nc.vector.tensor_copy(g_sb, g_acc[0:CQK + 1, 0:C + 2])

        cc_in = dram.tile([CQK + 1, C + 2], F32)
        cc_out = dram.tile([CQK + 1, C + 2], F32)
        if 1 in phases:
            nc.sync.dma_start(cc_in, g_sb)
        if use_collective and 1 in phases:
            nc.gpsimd.collective_compute(
                "AllReduce",
                mybir.AluOpType.add,
                replica_groups=groups,
                ins=[cc_in.opt()],
                outs=[cc_out.opt()],
            )
        elif 1 in phases:
            nc.sync.dma_start(cc_out, cc_in)

        if 2 not in phases:
            dummy = singles.tile([P, NT], BF16)
            nc.sync.dma_start(dummy, xs_r[:, 0, 0:NT])
            nc.sync.dma_start(out_r[:, 0, 0:NT], dummy)
            return

        # ------------- post-collective: kvp = G wv^T, ksum vector -------------
        gsb = singles.tile([CQK + 1, C + 2], F32)
        nc.sync.dma_start(gsb, cc_out)
        ksum_f = singles.tile([P, CQK], F32)
        nc.sync.dma_start(ksum_f,
                          cc_out[0:CQK, C:C + 1].partition_broadcast(P))
        ksum = singles.tile([P, CQK + 2], BF16)
        nc.vector.tensor_scalar_add(ksum[:, 0:CQK], ksum_f, EPS)
        nc.vector.memset(ksum[:, CQK:CQK + 1], float(n_total))

        gT_sb = singles.tile([P, 2, CQK + 2], BF16)
        kvp_sb = singles.tile([CQK + 1, C], BF16)
        with ExitStack() as pk:
            ps_k = pk.enter_context(tc.tile_pool(name="ps_k", bufs=2,
                                                 space="PSUM"))
            gT_ps = ps_k.tile([P, 2, CQK + 1], F32)
            for k in range(2):
                nc.tensor.transpose(gT_ps[:, k, :],
                                    gsb[0:CQK + 1, k * P:(k + 1) * P],
                                    ident_f[0:CQK + 1, 0:CQK + 1])
            nc.vector.tensor_copy(gT_sb[:, :, 0:CQK + 1], gT_ps)
            kvp_ps = ps_k.tile([CQK + 1, C], F32)
            for k in range(2):
                nc.tensor.matmul(
                    kvp_ps,
                    gT_sb[:, k, 0:CQK + 1],
                    wvt_sb[:, k, :],
                    start=(k == 0),
                    stop=(k == 1),
                )
            nc.vector.tensor_copy(kvp_sb, kvp_ps)

        # ---------------- phase 2: out = kvp^T ([Q; s] / den) ----------------
        with ExitStack() as p2:
            scr2 = p2.enter_context(tc.tile_pool(name="scr2", bufs=4))
            qtp = p2.enter_context(tc.tile_pool(name="qtp", bufs=4))
            outp = p2.enter_context(tc.tile_pool(name="outp", bufs=4))
            ps_qt = p2.enter_context(tc.tile_pool(name="ps_qt", bufs=3,
                                                  space="PSUM"))
            ps_out = p2.enter_context(tc.tile_pool(name="ps_out", bufs=5,
                                                   space="PSUM"))

            MP = ST  # one macro: [128, 8, .] rows, 1024 n
            for m in range(NM):
                st_sl = stash[:, m * MP:(m + 1) * MP, 0:CQK + 1]

                prod = scr2.tile([P, MP, CQK + 1], BF16, tag="prod")
                nc.gpsimd.tensor_tensor(
                    prod, st_sl,
                    ksum[:, None, 0:CQK + 1].to_broadcast((P, MP, CQK + 1)),
                    mult)
                den = scr2.tile([P, MP, 1], BF16, tag="den")
                with nc.allow_low_precision(reason="bf16 den sum"):
                    nc.vector.reduce_sum(den, prod, axis=mybir.AxisListType.X)
                d = scr2.tile([P, MP, 1], BF16, tag="d")
                with nc.allow_low_precision(reason="bf16 reciprocal"):
                    nc.vector.reciprocal(d, den)
                qsc = scr2.tile([P, MP, CQK + 2], BF16, tag="qsc")
                nc.gpsimd.tensor_tensor(
                    qsc[:, :, 0:CQK + 1], st_sl,
                    d.to_broadcast((P, MP, CQK + 1)), mult)

                qt_ps = ps_qt.tile([CQK + 1, MP, P], BF16)  # 1 bank
                for s in range(MP):
                    nc.tensor.transpose(qt_ps[:, s, :], qsc[:, s, 0:CQK + 1],
                                        ident_b)
                qt_sb = qtp.tile([CQK + 1, MP * P], BF16)
                nc.vector.tensor_copy(qt_sb, qt_ps)

                ot = outp.tile([P, 2, NT], BF16)
                for mh in range(2):
                    for blk in range(2):
                        o_ps = ps_out.tile([P, NT // 2], F32, tag="o_ps")
                        nc.tensor.matmul(
                            o_ps,
                            kvp_sb[:, blk * P:(blk + 1) * P],
                            qt_sb[:, mh * (NT // 2):(mh + 1) * (NT // 2)],
                            start=True,
                            stop=True,
                        )
                        dst = ot[:, blk, mh * (NT // 2):(mh + 1) * (NT // 2)]
                        if (mh, blk) == (0, 0):
                            nc.vector.tensor_copy(dst, o_ps)
                        else:
                            nc.scalar.copy(dst, o_ps)
                nc.sync.dma_start(out_r[:, :, m * NT:(m + 1) * NT], ot)


def build_attention_nc(nsh, n_total, num_cores, groups, repeat=1,
                       use_collective=True, phases=(1, 2)):
    nc = bacc.Bacc("TRN2", target_bir_lowering=False, debug=False,
                   num_devices=num_cores)
    xs = nc.dram_tensor("xs", [C, nsh], BF16, kind="ExternalInput").ap()
    wqk = nc.dram_tensor("wqk", [C, 2 * CQK], BF16, kind="ExternalInput").ap()
    wvt = nc.dram_tensor("wvt", [C, C], BF16, kind="ExternalInput").ap()
    out = nc.dram_tensor("out", [C, nsh], BF16, kind="ExternalOutput").ap()
    with tile.TileContext(nc) as tc:
        for _ in range(repeat):
            emit_attention(tc, xs, wqk, wvt, out, nsh, n_total, groups,
                           use_collective=use_collective, phases=phases)
    nc.compile()
    return nc


_NC_CACHE = {}


def _get_nc(nsh, n_total, num_cores, groups_key):
    key = (nsh, n_total, num_cores, groups_key)
    if key not in _NC_CACHE:
        groups = [list(g) for g in groups_key]
        _NC_CACHE[key] = build_attention_nc(nsh, n_total, num_cores, groups)
    return _NC_CACHE[key]


def make_in_maps(inputs, nsh):
    """Host-side prep: bf16 casts + per-core shards. inputs: full arrays."""
    import ml_dtypes
    bf = ml_dtypes.bfloat16
    x = np.asarray(inputs["x"])
    B, Cc, H, W = x.shape
    N = H * W
    spb = N // nsh  # shards per batch
    xr = x.reshape(B, Cc, N)
    wqk = np.ascontiguousarray(
        np.concatenate([np.asarray(inputs["wq"]).T,
                        np.asarray(inputs["wk"]).T], axis=1)).astype(bf)
    wvt = np.ascontiguousarray(np.asarray(inputs["wv"]).T).astype(bf)
    in_maps = []
    for core in range(B * spb):
        b, hh = core // spb, core % spb
        in_maps.append({
            "xs": np.ascontiguousarray(
                xr[b, :, hh * nsh:(hh + 1) * nsh]).astype(bf),
            "wqk": wqk,
            "wvt": wvt,
        })
    return in_maps


def _kernel_numpy(x, wq, bq, wk, bk, wv, bv):
    b, c, h, w = x.shape
    n = h * w
    xf = x.reshape(b, c, n).astype(np.float64)
    Q = np.einsum("oc,bcn->bon", wq.astype(np.float64), xf) + bq.astype(np.float64)[None, :, None]
    K = np.einsum("oc,bcn->bon", wk.astype(np.float64), xf) + bk.astype(np.float64)[None, :, None]
    V = np.einsum("oc,bcn->bon", wv.astype(np.float64), xf) + bv.astype(np.float64)[None, :, None]
    Qn = Q / np.linalg.norm(Q, axis=1, keepdims=True)
    Kn = K / np.linalg.norm(K, axis=1, keepdims=True)
    k_sum = Kn.sum(-1) + EPS
    tailor = 1.0 / (n + np.einsum("bmn,bm->bn", Qn, k_sum))
    value_sum = V.sum(-1)
    kv = np.einsum("bmn,bcn->bmc", Kn, V)
    ms = value_sum[:, :, None] + np.einsum("bmn,bmc->bcn", Qn, kv)
    return (ms * tailor[:, None, :]).reshape(b, c, h, w).astype(np.float32)


def kernel(x, wq, bq, wk, bk, wv, bv):
    x = np.asarray(x, dtype=np.float32)
    B, Cc, H, W = x.shape
    if (any(np.any(np.asarray(b_) != 0) for b_ in (bq, bk, bv))
            or Cc != C or wq.shape != (CQK, C) or wv.shape != (C, C)
            or (H * W) % (2 * NT) != 0 or B != 4):
        return _kernel_numpy(x, wq, bq, wk, bk, wv, bv)
    N = H * W
    ncores = 8
    spb = ncores // B  # 2
    nsh = N // spb
    groups_key = tuple(
        tuple(range(b * spb, (b + 1) * spb)) for b in range(B))

    nc = _get_nc(nsh, N, ncores, groups_key)
    in_maps = make_in_maps(
        {"x": x, "wq": wq, "wk": wk, "wv": wv}, nsh)
    res = run_bass_kernel_spmd(nc, in_maps, list(range(ncores)))

    out = np.empty((B, Cc, N), np.float32)
    for core in range(ncores):
        b, hh = core // spb, core % spb
        out[b, :, hh * nsh:(hh + 1) * nsh] = \
            np.asarray(res.results[core]["out"]).astype(np.float32)
    return out.reshape(B, Cc, H, W)


# revision 15
# speedup vs baseline: 5.2636x; 1.6058x over previous
"""Trainium2 Bass kernel for nn_Attention_54142357733562 (linear attention).

Reference math (per batch b, x flattened to [C, N]):
    Q = wq @ x ; K = wk @ x ; V = wv @ x
    Qn = Q / ||Q||_c ; Kn = K / ||K||_c
    k_sum = sum_n Kn + EPS
    out = (value_sum + kv^T Qn) / (N + Qn^T k_sum),  kv = Kn V^T

Algebraic restructure used here (all matmul inputs bf16):
    s[n] = ||Q[:, n]||;  den[n] = N*s[n] + Q[:, n].k_sum
    G'   = [Kn | 1]^T [x^T | 1]        # [33, 258]: G = Kn x^T, ksum, x_sum, n
    (AllReduce G' over the 2-core pair)
    kvp  = G'[:, 0:256] @ wv^T          # rows 0:32 = kv, row 32 = value_sum
    out[c, n] = sum_m [kvp][m, c] * ([Q; s][m, n] / den[n])
V is never materialized: kv == (Kn x^T) wv^T and value_sum == wv x_sum,
so phase 1 only computes Q, K (64 channels) plus a PE transpose of x.

Sharding: 8 cores = 4 batches x 2 N-halves. x is uploaded in bf16 (16 MiB
per core), out is written in bf16 and upcast on the host; HBM traffic is
half of the f32 version. The AllReduce payload is [33, 258] f32.
"""

import numpy as np
from contextlib import ExitStack

import concourse.bass as bass
import concourse.mybir as mybir
import concourse.tile as tile
from concourse import bacc
from concourse.bass_utils import run_bass_kernel_spmd
from concourse.masks import make_identity

F32 = mybir.dt.float32
BF16 = mybir.dt.bfloat16

C = 256
CQK = 32
P = 128
NT = 1024  # macro-tile width along N
ST = NT // P  # 8
SW = 66    # stash row: [Q 0:32 | s 32 | ||K|| 33 | K 34:66]
GW = 291   # g_in row: [Kn 0:32 | 1 | x^T 33:289 | 1 | 1]
EPS = 1e-6


def emit_attention(tc, xs, wqk, wvt, out, nsh, n_total, groups,
                   use_collective=True, phases=(1, 2)):
    nc = tc.nc
    NM = nsh // NT
    SROW = nsh // P

    xs_r = xs.rearrange("(o p) n -> p o n", p=P)    # [128, 2, nsh]
    out_r = out.rearrange("(o p) n -> p o n", p=P)
    wqk_r = wqk.rearrange("(o p) j -> p o j", p=P)  # [128, 2, 64]
    wvt_r = wvt.rearrange("(o p) c -> p o c", p=P)  # [128, 2, 256]

    mult = mybir.AluOpType.mult

    with ExitStack() as ctx:
        singles = ctx.enter_context(tc.tile_pool(name="singles", bufs=1))
        dram = ctx.enter_context(tc.tile_pool(name="dram", bufs=1, space="DRAM"))

        wqk_sb = singles.tile([P, 2, 2 * CQK], BF16)
        nc.sync.dma_start(wqk_sb, wqk_r)
        wvt_sb = singles.tile([P, 2, C], BF16)
        nc.sync.dma_start(wvt_sb, wvt_r)
        ident_f = singles.tile([P, P], F32)
        make_identity(nc, ident_f)
        ident_b = singles.tile([P, P], BF16)
        nc.vector.tensor_copy(ident_b, ident_f)

        stash = singles.tile([P, SROW, SW], BF16)

        # ---------------- phase 1: QK + x^T + G accumulation ----------------
        if 1 in phases:
            with ExitStack() as p1:
                xp = p1.enter_context(tc.tile_pool(name="xp", bufs=4))
                gi = p1.enter_context(tc.tile_pool(name="gi", bufs=3))
                scr = p1.enter_context(tc.tile_pool(name="scr", bufs=3))
                ps_qk = p1.enter_context(
                    tc.tile_pool(name="ps_qk", bufs=2, space="PSUM"))
                ps_xt = p1.enter_context(
                    tc.tile_pool(name="ps_xt", bufs=4, space="PSUM"))
                ps_g = p1.enter_context(
                    tc.tile_pool(name="ps_g", bufs=1, space="PSUM"))

                g_acc = ps_g.tile([P, 512], F32)  # rows 0:33, cols 0:258 used

                xdrain = 0
                for m in range(NM):
                    xt = xp.tile([P, 2, NT], BF16)
                    nc.sync.dma_start(xt, xs_r[:, :, m * NT:(m + 1) * NT])

                    gi_t = gi.tile([P, ST, GW], BF16)
                    nc.gpsimd.memset(gi_t[:, :, CQK:CQK + 1], 1.0)
                    nc.gpsimd.memset(gi_t[:, :, GW - 2:GW], 1.0)

                    qk_ps = ps_qk.tile([P, ST, 64], F32)  # 1 bank
                    for h in range(ST // 2):
                        xt_ps = ps_xt.tile([P, 2, 512], BF16)  # 1 bank
                        for s2 in range(2):
                            s = 2 * h + s2
                            ch = slice(s * P, (s + 1) * P)
                            for o in range(2):
                                nc.tensor.matmul(
                                    qk_ps[:, s, :],
                                    xt[:, o, ch],
                                    wqk_sb[:, o, :],
                                    start=(o == 0),
                                    stop=(o == 1),
                                )
                            for o in range(2):
                                nc.tensor.transpose(
                                    xt_ps[:, s2, o * P:(o + 1) * P],
                                    xt[:, o, ch],
                                    ident_b,
                                )
                        # x^T drain (bf16 -> bf16; 2x on DVE)
                        xt_dst = gi_t[:, 2 * h:2 * h + 2, CQK + 1:CQK + 1 + C]
                        if xdrain % 8 < 5:
                            nc.vector.tensor_copy(xt_dst, xt_ps[:, :, 0:C])
                        else:
                            nc.scalar.copy(xt_dst, xt_ps[:, :, 0:C])
                        xdrain += 1

                    # QK drain once per macro (strided into stash) on ACT
                    mst = stash[:, m * ST:(m + 1) * ST, :]
                    qk_dst = bass.AP(
                        tensor=mst.tensor,
                        offset=mst.offset,
                        ap=[mst.ap[0], mst.ap[1], [CQK + 2, 2], [1, CQK]],
                    )
                    nc.scalar.copy(qk_dst, qk_ps.rearrange("p h (g c) -> p h g c", g=2))

                    # norms: s = ||Q||, ||K||; Kn = K / ||K||
                    qk_view = bass.AP(
                        tensor=mst.tensor,
                        offset=mst.offset,
                        ap=[mst.ap[0], mst.ap[1], [CQK + 2, 2], [1, CQK]],
                    )
                    sq = scr.tile([P, ST, 2, CQK], BF16, tag="sq")
                    nc.vector.tensor_tensor(sq, qk_view, qk_view, mult)
                    ssq = scr.tile([P, ST, 2], BF16, tag="ssq")
                    with nc.allow_low_precision(reason="bf16 norm sums"):
                        nc.vector.reduce_sum(ssq, sq, axis=mybir.AxisListType.X)
                    nc.scalar.sqrt(mst[:, :, CQK:CQK + 2], ssq)
                    rk = scr.tile([P, ST, 1], F32, tag="rk")
                    nc.vector.reciprocal(rk, mst[:, :, CQK + 1:CQK + 2])
                    nc.gpsimd.tensor_tensor(gi_t[:, :, 0:CQK],
                                            mst[:, :, CQK + 2:SW],
                                            rk.to_broadcast((P, ST, CQK)), mult)

                    for s in range(ST):
                        nc.tensor.matmul(
                            g_acc[0:CQK + 1, 0:C + 2],
                            gi_t[:, s, 0:CQK + 1],
                            gi_t[:, s, CQK + 1:GW],
                            start=(m == 0 and s == 0),
                            stop=(m == NM - 1 and s == ST - 1),
                        )

                g_sb = singles.tile([CQK + 1, C + 2], F32)
                nc.vector.tensor_copy(g_sb, g_acc[0:CQK + 1, 0:C + 2])

        cc_in = dram.tile([CQK + 1, C + 2], F32)
        cc_out = dram.tile([2, CQK + 1, C + 2], F32)
        if 1 in phases:
            nc.sync.dma_start(cc_in, g_sb)
        if use_collective and 1 in phases:
            nc.gpsimd.collective_compute(
                "AllGather",
                mybir.AluOpType.bypass,
                replica_groups=groups,
                ins=[cc_in.opt()],
                outs=[cc_out.opt()],
            )
        elif 1 in phases:
            for k in range(2):
                nc.sync.dma_start(cc_out[k], cc_in)

        if 2 not in phases:
            # keep phase 1 live: route the collective result to 'out'
            res_sb = singles.tile([CQK + 1, C + 2], F32)
            nc.sync.dma_start(res_sb, cc_out[0])
            res_b = singles.tile([CQK + 1, C + 2], BF16)
            nc.vector.tensor_copy(res_b, res_sb)
            nc.sync.dma_start(out_r[0:CQK + 1, 0, 0:C + 2], res_b)
            return

        # ------------- post-collective: kvp = G wv^T, ksum vector -------------
        gsb2 = singles.tile([CQK + 1, 2, C + 2], F32)
        nc.sync.dma_start(gsb2, cc_out.rearrange("k m c -> m k c"))
        gsb = singles.tile([CQK + 1, C + 2], F32)
        nc.vector.tensor_tensor(gsb, gsb2[:, 0, :], gsb2[:, 1, :],
                                mybir.AluOpType.add)
        ksum_f = singles.tile([P, 2, CQK], F32)
        for k in range(2):
            nc.sync.dma_start(ksum_f[:, k, :],
                              cc_out[k, 0:CQK, C:C + 1].partition_broadcast(P))
        ksum_s = singles.tile([P, CQK], F32)
        nc.vector.tensor_tensor(ksum_s, ksum_f[:, 0, :], ksum_f[:, 1, :],
                                mybir.AluOpType.add)
        ksum = singles.tile([P, CQK + 2], BF16)
        nc.vector.tensor_scalar_add(ksum[:, 0:CQK], ksum_s, EPS)
        nc.vector.memset(ksum[:, CQK:CQK + 1], float(n_total))

        gT_sb = singles.tile([P, 2, CQK + 2], BF16)
        kvp_sb = singles.tile([CQK + 1, C], BF16)
        with ExitStack() as pk:
            ps_k = pk.enter_context(tc.tile_pool(name="ps_k", bufs=2,
                                                 space="PSUM"))
            gT_ps = ps_k.tile([P, 2, CQK + 1], F32)
            for k in range(2):
                nc.tensor.transpose(gT_ps[:, k, :],
                                    gsb[0:CQK + 1, k * P:(k + 1) * P],
                                    ident_f[0:CQK + 1, 0:CQK + 1])
            nc.vector.tensor_copy(gT_sb[:, :, 0:CQK + 1], gT_ps)
            kvp_ps = ps_k.tile([CQK + 1, C], F32)
            for k in range(2):
                nc.tensor.matmul(
                    kvp_ps,
                    gT_sb[:, k, 0:CQK + 1],
                    wvt_sb[:, k, :],
                    start=(k == 0),
                    stop=(k == 1),
                )
            nc.vector.tensor_copy(kvp_sb, kvp_ps)

        # ---------------- phase 2: out = kvp^T ([Q; s] / den) ----------------
        with ExitStack() as p2:
            scr2 = p2.enter_context(tc.tile_pool(name="scr2", bufs=4))
            qtp = p2.enter_context(tc.tile_pool(name="qtp", bufs=4))
            outp = p2.enter_context(tc.tile_pool(name="outp", bufs=4))
            ps_qt = p2.enter_context(tc.tile_pool(name="ps_qt", bufs=3,
                                                  space="PSUM"))
            ps_out = p2.enter_context(tc.tile_pool(name="ps_out", bufs=5,
                                                   space="PSUM"))

            MP = ST  # one macro: [128, 8, .] rows, 1024 n

            def emit_prod(m):
                st_sl = stash[:, m * MP:(m + 1) * MP, 0:CQK + 1]
                prod = scr2.tile([P, MP, CQK + 1], BF16, tag="prod")
                nc.gpsimd.tensor_tensor(
                    prod, st_sl,
                    ksum[:, None, 0:CQK + 1].to_broadcast((P, MP, CQK + 1)),
                    mult)
                return prod

            for m in range(NM):
                st_sl = stash[:, m * MP:(m + 1) * MP, 0:CQK + 1]

                prod = emit_prod(m)
                den = scr2.tile([P, MP, 1], BF16, tag="den")
                with nc.allow_low_precision(reason="bf16 den sum"):
                    nc.vector.reduce_sum(den, prod, axis=mybir.AxisListType.X)
                d = scr2.tile([P, MP, 1], BF16, tag="d")
                with nc.allow_low_precision(reason="bf16 reciprocal"):
                    nc.vector.reciprocal(d, den)
                qsc = scr2.tile([P, MP, CQK + 2], BF16, tag="qsc")
                nc.gpsimd.tensor_tensor(
                    qsc[:, :, 0:CQK + 1], st_sl,
                    d.to_broadcast((P, MP, CQK + 1)), mult)

                qt_ps = ps_qt.tile([CQK + 1, MP, P], BF16)  # 1 bank
                for s in range(MP):
                    nc.tensor.transpose(qt_ps[:, s, :], qsc[:, s, 0:CQK + 1],
                                        ident_b)
                qt_sb = qtp.tile([CQK + 1, MP * P], BF16)
                nc.vector.tensor_copy(qt_sb, qt_ps)

                ot = outp.tile([P, 2, NT], BF16)
                for mh in range(2):
                    for blk in range(2):
                        o_ps = ps_out.tile([P, NT // 2], F32, tag="o_ps")
                        nc.tensor.matmul(
                            o_ps,
                            kvp_sb[:, blk * P:(blk + 1) * P],
                            qt_sb[:, mh * (NT // 2):(mh + 1) * (NT // 2)],
                            start=True,
                            stop=True,
                        )
                        dst = ot[:, blk, mh * (NT // 2):(mh + 1) * (NT // 2)]
                        if (mh, blk) == (0, 0):
                            nc.vector.tensor_copy(dst, o_ps)
                        else:
                            nc.scalar.copy(dst, o_ps)
                nc.sync.dma_start(out_r[:, :, m * NT:(m + 1) * NT], ot)


def build_attention_nc(nsh, n_total, num_cores, groups, repeat=1,
                       use_collective=True, phases=(1, 2)):
    nc = bacc.Bacc("TRN2", target_bir_lowering=False, debug=False,
                   num_devices=num_cores)
    xs = nc.dram_tensor("xs", [C, nsh], BF16, kind="ExternalInput").ap()
    wqk = nc.dram_tensor("wqk", [C, 2 * CQK], BF16, kind="ExternalInput").ap()
    wvt = nc.dram_tensor("wvt", [C, C], BF16, kind="ExternalInput").ap()
    out = nc.dram_tensor("out", [C, nsh], BF16, kind="ExternalOutput").ap()
    with tile.TileContext(nc) as tc:
        for _ in range(repeat):
            emit_attention(tc, xs, wqk, wvt, out, nsh, n_total, groups,
                           use_collective=use_collective, phases=phases)
    nc.compile()
    return nc


_NC_CACHE = {}


def _get_nc(nsh, n_total, num_cores, groups_key):
    key = (nsh, n_total, num_cores, groups_key)
    if key not in _NC_CACHE:
        groups = [list(g) for g in groups_key]
        _NC_CACHE[key] = build_attention_nc(nsh, n_total, num_cores, groups)
    return _NC_CACHE[key]


def make_in_maps(inputs, nsh):
    """Host-side prep: bf16 casts + per-core shards. inputs: full arrays."""
    import ml_dtypes
    bf = ml_dtypes.bfloat16
    x = np.asarray(inputs["x"])
    B, Cc, H, W = x.shape
    N = H * W
    spb = N // nsh  # shards per batch
    xr = x.reshape(B, Cc, N)
    wqk = np.ascontiguousarray(
        np.concatenate([np.asarray(inputs["wq"]).T,
                        np.asarray(inputs["wk"]).T], axis=1)).astype(bf)
    wvt = np.ascontiguousarray(np.asarray(inputs["wv"]).T).astype(bf)
    in_maps = []
    for core in range(B * spb):
        b, hh = core // spb, core % spb
        in_maps.append({
            "xs": np.ascontiguousarray(
                xr[b, :, hh * nsh:(hh + 1) * nsh]).astype(bf),
            "wqk": wqk,
            "wvt": wvt,
        })
    return in_maps


def _kernel_numpy(x, wq, bq, wk, bk, wv, bv):
    b, c, h, w = x.shape
    n = h * w
    xf = x.reshape(b, c, n).astype(np.float64)
    Q = np.einsum("oc,bcn->bon", wq.astype(np.float64), xf) + bq.astype(np.float64)[None, :, None]
    K = np.einsum("oc,bcn->bon", wk.astype(np.float64), xf) + bk.astype(np.float64)[None, :, None]
    V = np.einsum("oc,bcn->bon", wv.astype(np.float64), xf) + bv.astype(np.float64)[None, :, None]
    Qn = Q / np.linalg.norm(Q, axis=1, keepdims=True)
    Kn = K / np.linalg.norm(K, axis=1, keepdims=True)
    k_sum = Kn.sum(-1) + EPS
    tailor = 1.0 / (n + np.einsum("bmn,bm->bn", Qn, k_sum))
    value_sum = V.sum(-1)
    kv = np.einsum("bmn,bcn->bmc", Kn, V)
    ms = value_sum[:, :, None] + np.einsum("bmn,bmc->bcn", Qn, kv)
    return (ms * tailor[:, None, :]).reshape(b, c, h, w).astype(np.float32)


def kernel(x, wq, bq, wk, bk, wv, bv):
    x = np.asarray(x, dtype=np.float32)
    B, Cc, H, W = x.shape
    if (any(np.any(np.asarray(b_) != 0) for b_ in (bq, bk, bv))
            or Cc != C or wq.shape != (CQK, C) or wv.shape != (C, C)
            or (H * W) % (2 * NT) != 0 or B != 4):
        return _kernel_numpy(x, wq, bq, wk, bk, wv, bv)
    N = H * W
    ncores = 8
    spb = ncores // B  # 2
    nsh = N // spb
    groups_key = tuple(
        tuple(range(b * spb, (b + 1) * spb)) for b in range(B))

    nc = _get_nc(nsh, N, ncores, groups_key)
    in_maps = make_in_maps(
        {"x": x, "wq": wq, "wk": wk, "wv": wv}, nsh)
    res = run_bass_kernel_spmd(nc, in_maps, list(range(ncores)))

    out = np.empty((B, Cc, N), np.float32)
    for core in range(ncores):
        b, hh = core // spb, core % spb
        out[b, :, hh * nsh:(hh + 1) * nsh] = \
            np.asarray(res.results[core]["out"]).astype(np.float32)
    return out.reshape(B, Cc, H, W)


# revision 17
# speedup vs baseline: 5.2902x; 1.0050x over previous
"""Trainium2 Bass kernel for nn_Attention_54142357733562 (linear attention).

Reference math (per batch b, x flattened to [C, N]):
    Q = wq @ x ; K = wk @ x ; V = wv @ x
    Qn = Q / ||Q||_c ; Kn = K / ||K||_c
    k_sum = sum_n Kn + EPS
    out = (value_sum + kv^T Qn) / (N + Qn^T k_sum),  kv = Kn V^T

Algebraic restructure used here (all matmul inputs bf16):
    s[n] = ||Q[:, n]||;  den[n] = N*s[n] + Q[:, n].k_sum
    G'   = [Kn | 1]^T [x^T | 1]        # [33, 258]: G = Kn x^T, ksum, x_sum, n
    (AllReduce G' over the 2-core pair)
    kvp  = G'[:, 0:256] @ wv^T          # rows 0:32 = kv, row 32 = value_sum
    out[c, n] = sum_m [kvp][m, c] * ([Q; s][m, n] / den[n])
V is never materialized: kv == (Kn x^T) wv^T and value_sum == wv x_sum,
so phase 1 only computes Q, K (64 channels) plus a PE transpose of x.

Sharding: 8 cores = 4 batches x 2 N-halves. x is uploaded in bf16 (16 MiB
per core), out is written in bf16 and upcast on the host; HBM traffic is
half of the f32 version. The AllReduce payload is [33, 258] f32.
"""

import numpy as np
from contextlib import ExitStack

import concourse.bass as bass
import concourse.mybir as mybir
import concourse.tile as tile
from concourse import bacc
from concourse.bass_utils import run_bass_kernel_spmd
from concourse.masks import make_identity

F32 = mybir.dt.float32
BF16 = mybir.dt.bfloat16

C = 256
CQK = 32
P = 128
NT = 1024  # macro-tile width along N
ST = NT // P  # 8
SW = 66    # stash row: [Q 0:32 | s 32 | ||K|| 33 | K 34:66]
GW = 291   # g_in row: [Kn 0:32 | 1 | x^T 33:289 | 1 | 1]
EPS = 1e-6


def emit_attention(tc, xs, wqk, wvt, out, nsh, n_total, groups,
                   use_collective=True, phases=(1, 2)):
    nc = tc.nc
    NM = nsh // NT
    SROW = nsh // P

    xs_r = xs.rearrange("(o p) n -> p o n", p=P)    # [128, 2, nsh]
    out_r = out.rearrange("(o p) n -> p o n", p=P)
    wqk_r = wqk.rearrange("(o p) j -> p o j", p=P)  # [128, 2, 64]
    wvt_r = wvt.rearrange("(o p) c -> p o c", p=P)  # [128, 2, 256]

    mult = mybir.AluOpType.mult

    with ExitStack() as ctx:
        singles = ctx.enter_context(tc.tile_pool(name="singles", bufs=1))
        dram = ctx.enter_context(tc.tile_pool(name="dram", bufs=1, space="DRAM"))

        wqk_sb = singles.tile([P, 2, 2 * CQK], BF16)
        nc.sync.dma_start(wqk_sb, wqk_r)
        wvt_sb = singles.tile([P, 2, C], BF16)
        nc.sync.dma_start(wvt_sb, wvt_r)
        ident_f = singles.tile([P, P], F32)
        make_identity(nc, ident_f)
        ident_b = singles.tile([P, P], BF16)
        nc.vector.tensor_copy(ident_b, ident_f)

        stash = singles.tile([P, SROW, SW], BF16)

        # ---------------- phase 1: QK + x^T + G accumulation ----------------
        if 1 in phases:
            with ExitStack() as p1:
                xp = p1.enter_context(tc.tile_pool(name="xp", bufs=4))
                gi = p1.enter_context(tc.tile_pool(name="gi", bufs=3))
                scr = p1.enter_context(tc.tile_pool(name="scr", bufs=3))
                ps_qk = p1.enter_context(
                    tc.tile_pool(name="ps_qk", bufs=2, space="PSUM"))
                ps_xt = p1.enter_context(
                    tc.tile_pool(name="ps_xt", bufs=5, space="PSUM"))
                ps_g = p1.enter_context(
                    tc.tile_pool(name="ps_g", bufs=1, space="PSUM"))

                g_acc = ps_g.tile([P, 512], F32)  # rows 0:33, cols 0:258 used

                xdrain = 0
                for m in range(NM):
                    xt = xp.tile([P, 2, NT], BF16)
                    nc.sync.dma_start(xt, xs_r[:, :, m * NT:(m + 1) * NT])

                    gi_t = gi.tile([P, ST, GW], BF16)
                    nc.gpsimd.memset(gi_t[:, :, CQK:CQK + 1], 1.0)
                    nc.gpsimd.memset(gi_t[:, :, GW - 2:GW], 1.0)

                    qk_ps = ps_qk.tile([P, ST, 64], F32)  # 1 bank
                    for h in range(ST // 2):
                        xt_ps = ps_xt.tile([P, 2, 512], BF16)  # 1 bank
                        for s2 in range(2):
                            s = 2 * h + s2
                            ch = slice(s * P, (s + 1) * P)
                            for o in range(2):
                                nc.tensor.matmul(
                                    qk_ps[:, s, :],
                                    xt[:, o, ch],
                                    wqk_sb[:, o, :],
                                    start=(o == 0),
                                    stop=(o == 1),
                                )
                            for o in range(2):
                                nc.tensor.transpose(
                                    xt_ps[:, s2, o * P:(o + 1) * P],
                                    xt[:, o, ch],
                                    ident_b,
                                )
                        # x^T drain (bf16 -> bf16; 2x on DVE)
                        xt_dst = gi_t[:, 2 * h:2 * h + 2, CQK + 1:CQK + 1 + C]
                        if xdrain % 8 < 5:
                            nc.vector.tensor_copy(xt_dst, xt_ps[:, :, 0:C])
                        else:
                            nc.scalar.copy(xt_dst, xt_ps[:, :, 0:C])
                        xdrain += 1

                    # QK drain once per macro (strided into stash) on ACT
                    mst = stash[:, m * ST:(m + 1) * ST, :]
                    qk_dst = bass.AP(
                        tensor=mst.tensor,
                        offset=mst.offset,
                        ap=[mst.ap[0], mst.ap[1], [CQK + 2, 2], [1, CQK]],
                    )
                    nc.scalar.copy(qk_dst, qk_ps.rearrange("p h (g c) -> p h g c", g=2))

                    # norms: s = ||Q||, ||K||; Kn = K / ||K||
                    qk_view = bass.AP(
                        tensor=mst.tensor,
                        offset=mst.offset,
                        ap=[mst.ap[0], mst.ap[1], [CQK + 2, 2], [1, CQK]],
                    )
                    sq = scr.tile([P, ST, 2, CQK], BF16, tag="sq")
                    nc.vector.tensor_tensor(sq, qk_view, qk_view, mult)
                    ssq = scr.tile([P, ST, 2], BF16, tag="ssq")
                    with nc.allow_low_precision(reason="bf16 norm sums"):
                        nc.vector.reduce_sum(ssq, sq, axis=mybir.AxisListType.X)
                    nc.scalar.sqrt(mst[:, :, CQK:CQK + 2], ssq)
                    rk = scr.tile([P, ST, 1], F32, tag="rk")
                    nc.vector.reciprocal(rk, mst[:, :, CQK + 1:CQK + 2])
                    nc.gpsimd.tensor_tensor(gi_t[:, :, 0:CQK],
                                            mst[:, :, CQK + 2:SW],
                                            rk.to_broadcast((P, ST, CQK)), mult)

                    for s in range(ST):
                        nc.tensor.matmul(
                            g_acc[0:CQK + 1, 0:C + 2],
                            gi_t[:, s, 0:CQK + 1],
                            gi_t[:, s, CQK + 1:GW],
                            start=(m == 0 and s == 0),
                            stop=(m == NM - 1 and s == ST - 1),
                        )

                g_sb = singles.tile([CQK + 1, C + 2], F32)
                nc.vector.tensor_copy(g_sb, g_acc[0:CQK + 1, 0:C + 2])

        cc_in = dram.tile([CQK + 1, C + 2], F32)
        cc_out = dram.tile([2, CQK + 1, C + 2], F32)
        if 1 in phases:
            nc.sync.dma_start(cc_in, g_sb)
        if use_collective and 1 in phases:
            nc.gpsimd.collective_compute(
                "AllGather",
                mybir.AluOpType.bypass,
                replica_groups=groups,
                ins=[cc_in.opt()],
                outs=[cc_out.opt()],
            )
        elif 1 in phases:
            for k in range(2):
                nc.sync.dma_start(cc_out[k], cc_in)

        if 2 not in phases:
            # keep phase 1 live: route the collective result to 'out'
            res_sb = singles.tile([CQK + 1, C + 2], F32)
            nc.sync.dma_start(res_sb, cc_out[0])
            res_b = singles.tile([CQK + 1, C + 2], BF16)
            nc.vector.tensor_copy(res_b, res_sb)
            nc.sync.dma_start(out_r[0:CQK + 1, 0, 0:C + 2], res_b)
            return

        # ------------- post-collective: ksum vector first, kvp = G wv^T -------------
        ksum_f = singles.tile([P, 2, CQK], F32)
        for k in range(2):
            nc.sync.dma_start(ksum_f[:, k, :],
                              cc_out[k, 0:CQK, C:C + 1].partition_broadcast(P))
        ksum_s = singles.tile([P, CQK], F32)
        nc.vector.tensor_tensor(ksum_s, ksum_f[:, 0, :], ksum_f[:, 1, :],
                                mybir.AluOpType.add)
        ksum = singles.tile([P, CQK + 2], BF16)
        nc.vector.tensor_scalar_add(ksum[:, 0:CQK], ksum_s, EPS)
        nc.vector.memset(ksum[:, CQK:CQK + 1], float(n_total))
        gsb2 = singles.tile([CQK + 1, 2, C + 2], F32)
        nc.sync.dma_start(gsb2, cc_out.rearrange("k m c -> m k c"))
        gsb = singles.tile([CQK + 1, C + 2], F32)
        nc.vector.tensor_tensor(gsb, gsb2[:, 0, :], gsb2[:, 1, :],
                                mybir.AluOpType.add)

        gT_sb = singles.tile([P, 2, CQK + 2], BF16)
        kvp_sb = singles.tile([CQK + 1, C], BF16)
        with ExitStack() as pk:
            ps_k = pk.enter_context(tc.tile_pool(name="ps_k", bufs=2,
                                                 space="PSUM"))
            gT_ps = ps_k.tile([P, 2, CQK + 1], F32)
            for k in range(2):
                nc.tensor.transpose(gT_ps[:, k, :],
                                    gsb[0:CQK + 1, k * P:(k + 1) * P],
                                    ident_f[0:CQK + 1, 0:CQK + 1])
            nc.vector.tensor_copy(gT_sb[:, :, 0:CQK + 1], gT_ps)
            kvp_ps = ps_k.tile([CQK + 1, C], F32)
            for k in range(2):
                nc.tensor.matmul(
                    kvp_ps,
                    gT_sb[:, k, 0:CQK + 1],
                    wvt_sb[:, k, :],
                    start=(k == 0),
                    stop=(k == 1),
                )
            nc.vector.tensor_copy(kvp_sb, kvp_ps)

        # ---------------- phase 2: out = kvp^T ([Q; s] / den) ----------------
        with ExitStack() as p2:
            scr2 = p2.enter_context(tc.tile_pool(name="scr2", bufs=4))
            qtp = p2.enter_context(tc.tile_pool(name="qtp", bufs=4))
            outp = p2.enter_context(tc.tile_pool(name="outp", bufs=5))
            ps_qt = p2.enter_context(tc.tile_pool(name="ps_qt", bufs=3,
                                                  space="PSUM"))
            ps_out = p2.enter_context(tc.tile_pool(name="ps_out", bufs=5,
                                                   space="PSUM"))

            MP = ST  # one macro: [128, 8, .] rows, 1024 n

            def emit_prod(m):
                st_sl = stash[:, m * MP:(m + 1) * MP, 0:CQK + 1]
                prod = scr2.tile([P, MP, CQK + 1], BF16, tag="prod")
                nc.gpsimd.tensor_tensor(
                    prod, st_sl,
                    ksum[:, None, 0:CQK + 1].to_broadcast((P, MP, CQK + 1)),
                    mult)
                return prod

            for m in range(NM):
                st_sl = stash[:, m * MP:(m + 1) * MP, 0:CQK + 1]

                prod = emit_prod(m)
                den = scr2.tile([P, MP, 1], BF16, tag="den")
                with nc.allow_low_precision(reason="bf16 den sum"):
                    nc.vector.reduce_sum(den, prod, axis=mybir.AxisListType.X)
                d = scr2.tile([P, MP, 1], BF16, tag="d")
                with nc.allow_low_precision(reason="bf16 reciprocal"):
                    nc.vector.reciprocal(d, den)
                qsc = scr2.tile([P, MP, CQK + 2], BF16, tag="qsc")
                nc.gpsimd.tensor_tensor(
                    qsc[:, :, 0:CQK + 1], st_sl,
                    d.to_broadcast((P, MP, CQK + 1)), mult)

                qt_ps = ps_qt.tile([CQK + 1, MP, P], BF16)  # 1 bank
                for s in range(MP):
                    nc.tensor.transpose(qt_ps[:, s, :], qsc[:, s, 0:CQK + 1],
                                        ident_b)
                qt_sb = qtp.tile([CQK + 1, MP * P], BF16)
                nc.vector.tensor_copy(qt_sb, qt_ps)

                ot = outp.tile([P, 2, NT], BF16)
                for mh in range(2):
                    for blk in range(2):
                        o_ps = ps_out.tile([P, NT // 2], F32, tag="o_ps")
                        nc.tensor.matmul(
                            o_ps,
                            kvp_sb[:, blk * P:(blk + 1) * P],
                            qt_sb[:, mh * (NT // 2):(mh + 1) * (NT // 2)],
                            start=True,
                            stop=True,
                        )
                        dst = ot[:, blk, mh * (NT // 2):(mh + 1) * (NT // 2)]
                        if (mh, blk) == (0, 0):
                            nc.vector.tensor_copy(dst, o_ps)
                        else:
                            nc.scalar.copy(dst, o_ps)
                nc.sync.dma_start(out_r[:, :, m * NT:(m + 1) * NT], ot)


def build_attention_nc(nsh, n_total, num_cores, groups, repeat=1,
                       use_collective=True, phases=(1, 2)):
    nc = bacc.Bacc("TRN2", target_bir_lowering=False, debug=False,
                   num_devices=num_cores)
    xs = nc.dram_tensor("xs", [C, nsh], BF16, kind="ExternalInput").ap()
    wqk = nc.dram_tensor("wqk", [C, 2 * CQK], BF16, kind="ExternalInput").ap()
    wvt = nc.dram_tensor("wvt", [C, C], BF16, kind="ExternalInput").ap()
    out = nc.dram_tensor("out", [C, nsh], BF16, kind="ExternalOutput").ap()
    with tile.TileContext(nc) as tc:
        for _ in range(repeat):
            emit_attention(tc, xs, wqk, wvt, out, nsh, n_total, groups,
                           use_collective=use_collective, phases=phases)
    nc.compile()
    return nc


_NC_CACHE = {}


def _get_nc(nsh, n_total, num_cores, groups_key):
    key = (nsh, n_total, num_cores, groups_key)
    if key not in _NC_CACHE:
        groups = [list(g) for g in groups_key]
        _NC_CACHE[key] = build_attention_nc(nsh, n_total, num_cores, groups)
    return _NC_CACHE[key]


def make_in_maps(inputs, nsh):
    """Host-side prep: bf16 casts + per-core shards. inputs: full arrays."""
    import ml_dtypes
    bf = ml_dtypes.bfloat16
    x = np.asarray(inputs["x"])
    B, Cc, H, W = x.shape
    N = H * W
    spb = N // nsh  # shards per batch
    xr = x.reshape(B, Cc, N)
    wqk = np.ascontiguousarray(
        np.concatenate([np.asarray(inputs["wq"]).T,
                        np.asarray(inputs["wk"]).T], axis=1)).astype(bf)
    wvt = np.ascontiguousarray(np.asarray(inputs["wv"]).T).astype(bf)
    in_maps = []
    for core in range(B * spb):
        b, hh = core // spb, core % spb
        in_maps.append({
            "xs": np.ascontiguousarray(
                xr[b, :, hh * nsh:(hh + 1) * nsh]).astype(bf),
            "wqk": wqk,
            "wvt": wvt,
        })
    return in_maps


def _kernel_numpy(x, wq, bq, wk, bk, wv, bv):
    b, c, h, w = x.shape
    n = h * w
    xf = x.reshape(b, c, n).astype(np.float64)
    Q = np.einsum("oc,bcn->bon", wq.astype(np.float64), xf) + bq.astype(np.float64)[None, :, None]
    K = np.einsum("oc,bcn->bon", wk.astype(np.float64), xf) + bk.astype(np.float64)[None, :, None]
    V = np.einsum("oc,bcn->bon", wv.astype(np.float64), xf) + bv.astype(np.float64)[None, :, None]
    Qn = Q / np.linalg.norm(Q, axis=1, keepdims=True)
    Kn = K / np.linalg.norm(K, axis=1, keepdims=True)
    k_sum = Kn.sum(-1) + EPS
    tailor = 1.0 / (n + np.einsum("bmn,bm->bn", Qn, k_sum))
    value_sum = V.sum(-1)
    kv = np.einsum("bmn,bcn->bmc", Kn, V)
    ms = value_sum[:, :, None] + np.einsum("bmn,bmc->bcn", Qn, kv)
    return (ms * tailor[:, None, :]).reshape(b, c, h, w).astype(np.float32)


def kernel(x, wq, bq, wk, bk, wv, bv):
    x = np.asarray(x, dtype=np.float32)
    B, Cc, H, W = x.shape
    if (any(np.any(np.asarray(b_) != 0) for b_ in (bq, bk, bv))
            or Cc != C or wq.shape != (CQK, C) or wv.shape != (C, C)
            or (H * W) % (2 * NT) != 0 or B != 4):
        return _kernel_numpy(x, wq, bq, wk, bk, wv, bv)
    N = H * W
    ncores = 8
    spb = ncores // B  # 2
    nsh = N // spb
    groups_key = tuple(
        tuple(range(b * spb, (b + 1) * spb)) for b in range(B))

    nc = _get_nc(nsh, N, ncores, groups_key)
    in_maps = make_in_maps(
        {"x": x, "wq": wq, "wk": wk, "wv": wv}, nsh)
    res = run_bass_kernel_spmd(nc, in_maps, list(range(ncores)))

    out = np.empty((B, Cc, N), np.float32)
    for core in range(ncores):
        b, hh = core // spb, core % spb
        out[b, :, hh * nsh:(hh + 1) * nsh] = \
            np.asarray(res.results[core]["out"]).astype(np.float32)
    return out.reshape(B, Cc, H, W)


# revision 19
# speedup vs baseline: 5.2982x; 1.0015x over previous
"""Trainium2 Bass kernel for nn_Attention_54142357733562 (linear attention).

Reference math (per batch b, x flattened to [C, N]):
    Q = wq @ x ; K = wk @ x ; V = wv @ x
    Qn = Q / ||Q||_c ; Kn = K / ||K||_c
    k_sum = sum_n Kn + EPS
    out = (value_sum + kv^T Qn) / (N + Qn^T k_sum),  kv = Kn V^T

Algebraic restructure used here (all matmul inputs bf16):
    s[n] = ||Q[:, n]||;  den[n] = N*s[n] + Q[:, n].k_sum
    G'   = [Kn | 1]^T [x^T | 1]        # [33, 258]: G = Kn x^T, ksum, x_sum, n
    (AllGather G' over the 2-core pair + local add == AllReduce, but cheaper)
    kvp  = G'[:, 0:256] @ wv^T          # rows 0:32 = kv, row 32 = value_sum
    out[c, n] = sum_m [kvp][m, c] * ([Q; s][m, n] / den[n])
V is never materialized: kv == (Kn x^T) wv^T and value_sum == wv x_sum,
so phase 1 only computes Q, K (64 channels) plus a PE transpose of x.

Sharding: 8 cores = 4 batches x 2 N-halves. x is uploaded in bf16 (16 MiB
per core), out is written in bf16 and upcast on the host; HBM traffic is
half of the f32 version. The AllReduce payload is [33, 258] f32.
"""

import numpy as np
from contextlib import ExitStack

import concourse.bass as bass
import concourse.mybir as mybir
import concourse.tile as tile
from concourse import bacc
from concourse.bass_utils import run_bass_kernel_spmd
from concourse.masks import make_identity

F32 = mybir.dt.float32
BF16 = mybir.dt.bfloat16

C = 256
CQK = 32
P = 128
NT = 1024  # macro-tile width along N
ST = NT // P  # 8
SW = 66    # stash row: [Q 0:32 | s 32 | ||K|| 33 | K 34:66]
GW = 291   # g_in row: [Kn 0:32 | 1 | x^T 33:289 | 1 | 1]
EPS = 1e-6


def emit_attention(tc, xs, wqk, wvt, out, nsh, n_total, groups,
                   use_collective=True, phases=(1, 2)):
    nc = tc.nc
    NM = nsh // NT
    SROW = nsh // P

    xs_r = xs.rearrange("(o p) n -> p o n", p=P)    # [128, 2, nsh]
    out_r = out.rearrange("(o p) n -> p o n", p=P)
    wqk_r = wqk.rearrange("(o p) j -> p o j", p=P)  # [128, 2, 64]
    wvt_r = wvt.rearrange("(o p) c -> p o c", p=P)  # [128, 2, 256]

    mult = mybir.AluOpType.mult

    with ExitStack() as ctx:
        singles = ctx.enter_context(tc.tile_pool(name="singles", bufs=1))
        dram = ctx.enter_context(tc.tile_pool(name="dram", bufs=1, space="DRAM"))

        wqk_sb = singles.tile([P, 2, 2 * CQK], BF16)
        nc.sync.dma_start(wqk_sb, wqk_r)
        wvt_sb = singles.tile([P, 2, C], BF16)
        nc.sync.dma_start(wvt_sb, wvt_r)
        ident_f = singles.tile([P, P], F32)
        make_identity(nc, ident_f)
        ident_b = singles.tile([P, P], BF16)
        nc.vector.tensor_copy(ident_b, ident_f)

        stash = singles.tile([P, SROW, SW], BF16)

        # ---------------- phase 1: QK + x^T + G accumulation ----------------
        if 1 in phases:
            with ExitStack() as p1:
                xp = p1.enter_context(tc.tile_pool(name="xp", bufs=4))
                gi = p1.enter_context(tc.tile_pool(name="gi", bufs=4))
                scr = p1.enter_context(tc.tile_pool(name="scr", bufs=4))
                ps_qk = p1.enter_context(
                    tc.tile_pool(name="ps_qk", bufs=2, space="PSUM"))
                ps_xt = p1.enter_context(
                    tc.tile_pool(name="ps_xt", bufs=5, space="PSUM"))
                ps_g = p1.enter_context(
                    tc.tile_pool(name="ps_g", bufs=1, space="PSUM"))

                g_acc = ps_g.tile([P, 512], F32)  # rows 0:33, cols 0:258 used

                xdrain = 0
                for m in range(NM):
                    xt = xp.tile([P, 2, NT], BF16)
                    nc.sync.dma_start(xt, xs_r[:, :, m * NT:(m + 1) * NT])

                    gi_t = gi.tile([P, ST, GW], BF16)
                    nc.gpsimd.memset(gi_t[:, :, CQK:CQK + 1], 1.0)
                    nc.gpsimd.memset(gi_t[:, :, GW - 2:GW], 1.0)

                    qk_ps = ps_qk.tile([P, ST, 64], F32)  # 1 bank
                    for h in range(ST // 2):
                        xt_ps = ps_xt.tile([P, 2, 512], BF16)  # 1 bank
                        for s2 in range(2):
                            s = 2 * h + s2
                            ch = slice(s * P, (s + 1) * P)
                            for o in range(2):
                                nc.tensor.matmul(
                                    qk_ps[:, s, :],
                                    xt[:, o, ch],
                                    wqk_sb[:, o, :],
                                    start=(o == 0),
                                    stop=(o == 1),
                                )
                            for o in range(2):
                                nc.tensor.transpose(
                                    xt_ps[:, s2, o * P:(o + 1) * P],
                                    xt[:, o, ch],
                                    ident_b,
                                )
                        # x^T drain (bf16 -> bf16; 2x on DVE)
                        xt_dst = gi_t[:, 2 * h:2 * h + 2, CQK + 1:CQK + 1 + C]
                        if xdrain % 8 < 5:
                            nc.vector.tensor_copy(xt_dst, xt_ps[:, :, 0:C])
                        else:
                            nc.scalar.copy(xt_dst, xt_ps[:, :, 0:C])
                        xdrain += 1

                    # QK drain once per macro (strided into stash) on ACT
                    mst = stash[:, m * ST:(m + 1) * ST, :]
                    qk_dst = bass.AP(
                        tensor=mst.tensor,
                        offset=mst.offset,
                        ap=[mst.ap[0], mst.ap[1], [CQK + 2, 2], [1, CQK]],
                    )
                    nc.scalar.copy(qk_dst, qk_ps.rearrange("p h (g c) -> p h g c", g=2))

                    # norms: s = ||Q||, ||K||; Kn = K / ||K||
                    qk_view = bass.AP(
                        tensor=mst.tensor,
                        offset=mst.offset,
                        ap=[mst.ap[0], mst.ap[1], [CQK + 2, 2], [1, CQK]],
                    )
                    sq = scr.tile([P, ST, 2, CQK], BF16, tag="sq")
                    nc.vector.tensor_tensor(sq, qk_view, qk_view, mult)
                    ssq = scr.tile([P, ST, 2], BF16, tag="ssq")
                    with nc.allow_low_precision(reason="bf16 norm sums"):
                        nc.vector.reduce_sum(ssq, sq, axis=mybir.AxisListType.X)
                    nc.scalar.sqrt(mst[:, :, CQK:CQK + 2], ssq)
                    rk = scr.tile([P, ST, 1], F32, tag="rk")
                    nc.vector.reciprocal(rk, mst[:, :, CQK + 1:CQK + 2])
                    nc.gpsimd.tensor_tensor(gi_t[:, :, 0:CQK],
                                            mst[:, :, CQK + 2:SW],
                                            rk.to_broadcast((P, ST, CQK)), mult)

                    for s in range(ST):
                        nc.tensor.matmul(
                            g_acc[0:CQK + 1, 0:C + 2],
                            gi_t[:, s, 0:CQK + 1],
                            gi_t[:, s, CQK + 1:GW],
                            start=(m == 0 and s == 0),
                            stop=(m == NM - 1 and s == ST - 1),
                        )

                g_sb = singles.tile([CQK + 1, C + 2], F32)
                nc.vector.tensor_copy(g_sb, g_acc[0:CQK + 1, 0:C + 2])

        cc_in = dram.tile([CQK + 1, C + 2], F32)
        cc_out = dram.tile([2, CQK + 1, C + 2], F32)
        if 1 in phases:
            nc.sync.dma_start(cc_in, g_sb)
        if use_collective and 1 in phases:
            nc.gpsimd.collective_compute(
                "AllGather",
                mybir.AluOpType.bypass,
                replica_groups=groups,
                ins=[cc_in.opt()],
                outs=[cc_out.opt()],
            )
        elif 1 in phases:
            for k in range(2):
                nc.sync.dma_start(cc_out[k], cc_in)

        if 2 not in phases:
            # keep phase 1 live: route the collective result to 'out'
            res_sb = singles.tile([CQK + 1, C + 2], F32)
            nc.sync.dma_start(res_sb, cc_out[0])
            res_b = singles.tile([CQK + 1, C + 2], BF16)
            nc.vector.tensor_copy(res_b, res_sb)
            nc.sync.dma_start(out_r[0:CQK + 1, 0, 0:C + 2], res_b)
            return

        # ------------- post-collective: ksum vector first, kvp = G wv^T -------------
        ksum_f = singles.tile([P, 2, CQK], F32)
        for k in range(2):
            nc.sync.dma_start(ksum_f[:, k, :],
                              cc_out[k, 0:CQK, C:C + 1].partition_broadcast(P))
        ksum_s = singles.tile([P, CQK], F32)
        nc.vector.tensor_tensor(ksum_s, ksum_f[:, 0, :], ksum_f[:, 1, :],
                                mybir.AluOpType.add)
        ksum = singles.tile([P, CQK + 2], BF16)
        nc.vector.tensor_scalar_add(ksum[:, 0:CQK], ksum_s, EPS)
        nc.vector.memset(ksum[:, CQK:CQK + 1], float(n_total))
        gsb2 = singles.tile([CQK + 1, 2, C + 2], F32)
        nc.sync.dma_start(gsb2, cc_out.rearrange("k m c -> m k c"))
        gsb = singles.tile([CQK + 1, C + 2], F32)
        nc.vector.tensor_tensor(gsb, gsb2[:, 0, :], gsb2[:, 1, :],
                                mybir.AluOpType.add)

        gT_sb = singles.tile([P, 2, CQK + 2], BF16)
        kvp_sb = singles.tile([CQK + 1, C], BF16)
        with ExitStack() as pk:
            ps_k = pk.enter_context(tc.tile_pool(name="ps_k", bufs=2,
                                                 space="PSUM"))
            gT_ps = ps_k.tile([P, 2, CQK + 1], F32)
            for k in range(2):
                nc.tensor.transpose(gT_ps[:, k, :],
                                    gsb[0:CQK + 1, k * P:(k + 1) * P],
                                    ident_f[0:CQK + 1, 0:CQK + 1])
            nc.vector.tensor_copy(gT_sb[:, :, 0:CQK + 1], gT_ps)
            kvp_ps = ps_k.tile([CQK + 1, C], F32)
            for k in range(2):
                nc.tensor.matmul(
                    kvp_ps,
                    gT_sb[:, k, 0:CQK + 1],
                    wvt_sb[:, k, :],
                    start=(k == 0),
                    stop=(k == 1),
                )
            nc.vector.tensor_copy(kvp_sb, kvp_ps)

        # ---------------- phase 2: out = kvp^T ([Q; s] / den) ----------------
        with ExitStack() as p2:
            scr2 = p2.enter_context(tc.tile_pool(name="scr2", bufs=4))
            qtp = p2.enter_context(tc.tile_pool(name="qtp", bufs=4))
            outp = p2.enter_context(tc.tile_pool(name="outp", bufs=5))
            ps_qt = p2.enter_context(tc.tile_pool(name="ps_qt", bufs=3,
                                                  space="PSUM"))
            ps_out = p2.enter_context(tc.tile_pool(name="ps_out", bufs=5,
                                                   space="PSUM"))

            MP = ST  # one macro: [128, 8, .] rows, 1024 n

            def emit_prod(m):
                st_sl = stash[:, m * MP:(m + 1) * MP, 0:CQK + 1]
                prod = scr2.tile([P, MP, CQK + 1], BF16, tag="prod")
                nc.gpsimd.tensor_tensor(
                    prod, st_sl,
                    ksum[:, None, 0:CQK + 1].to_broadcast((P, MP, CQK + 1)),
                    mult)
                return prod

            for m in range(NM):
                st_sl = stash[:, m * MP:(m + 1) * MP, 0:CQK + 1]

                prod = emit_prod(m)
                den = scr2.tile([P, MP, 1], BF16, tag="den")
                with nc.allow_low_precision(reason="bf16 den sum"):
                    nc.vector.reduce_sum(den, prod, axis=mybir.AxisListType.X)
                d = scr2.tile([P, MP, 1], BF16, tag="d")
                with nc.allow_low_precision(reason="bf16 reciprocal"):
                    nc.vector.reciprocal(d, den)
                qsc = scr2.tile([P, MP, CQK + 2], BF16, tag="qsc")
                nc.gpsimd.tensor_tensor(
                    qsc[:, :, 0:CQK + 1], st_sl,
                    d.to_broadcast((P, MP, CQK + 1)), mult)

                qt_ps = ps_qt.tile([CQK + 1, MP, P], BF16)  # 1 bank
                for s in range(MP):
                    nc.tensor.transpose(qt_ps[:, s, :], qsc[:, s, 0:CQK + 1],
                                        ident_b)
                qt_sb = qtp.tile([CQK + 1, MP * P], BF16)
                nc.vector.tensor_copy(qt_sb, qt_ps)

                ot = outp.tile([P, 2, NT], BF16)
                for mh in range(2):
                    for blk in range(2):
                        o_ps = ps_out.tile([P, NT // 2], F32, tag="o_ps")
                        nc.tensor.matmul(
                            o_ps,
                            kvp_sb[:, blk * P:(blk + 1) * P],
                            qt_sb[:, mh * (NT // 2):(mh + 1) * (NT // 2)],
                            start=True,
                            stop=True,
                        )
                        dst = ot[:, blk, mh * (NT // 2):(mh + 1) * (NT // 2)]
                        if (mh, blk) == (0, 0):
                            nc.vector.tensor_copy(dst, o_ps)
                        else:
                            nc.scalar.copy(dst, o_ps)
                nc.sync.dma_start(out_r[:, :, m * NT:(m + 1) * NT], ot)


def build_attention_nc(nsh, n_total, num_cores, groups, repeat=1,
                       use_collective=True, phases=(1, 2)):
    nc = bacc.Bacc("TRN2", target_bir_lowering=False, debug=False,
                   num_devices=num_cores)
    xs = nc.dram_tensor("xs", [C, nsh], BF16, kind="ExternalInput").ap()
    wqk = nc.dram_tensor("wqk", [C, 2 * CQK], BF16, kind="ExternalInput").ap()
    wvt = nc.dram_tensor("wvt", [C, C], BF16, kind="ExternalInput").ap()
    out = nc.dram_tensor("out", [C, nsh], BF16, kind="ExternalOutput").ap()
    with tile.TileContext(nc) as tc:
        for _ in range(repeat):
            emit_attention(tc, xs, wqk, wvt, out, nsh, n_total, groups,
                           use_collective=use_collective, phases=phases)
    nc.compile()
    return nc


_NC_CACHE = {}


def _get_nc(nsh, n_total, num_cores, groups_key):
    key = (nsh, n_total, num_cores, groups_key)
    if key not in _NC_CACHE:
        groups = [list(g) for g in groups_key]
        _NC_CACHE[key] = build_attention_nc(nsh, n_total, num_cores, groups)
    return _NC_CACHE[key]


def make_in_maps(inputs, nsh):
    """Host-side prep: bf16 casts + per-core shards. inputs: full arrays."""
    import ml_dtypes
    bf = ml_dtypes.bfloat16
    x = np.asarray(inputs["x"])
    B, Cc, H, W = x.shape
    N = H * W
    spb = N // nsh  # shards per batch
    xr = x.reshape(B, Cc, N)
    wqk = np.ascontiguousarray(
        np.concatenate([np.asarray(inputs["wq"]).T,
                        np.asarray(inputs["wk"]).T], axis=1)).astype(bf)
    wvt = np.ascontiguousarray(np.asarray(inputs["wv"]).T).astype(bf)
    in_maps = []
    for core in range(B * spb):
        b, hh = core // spb, core % spb
        in_maps.append({
            "xs": np.ascontiguousarray(
                xr[b, :, hh * nsh:(hh + 1) * nsh]).astype(bf),
            "wqk": wqk,
            "wvt": wvt,
        })
    return in_maps


def _kernel_numpy(x, wq, bq, wk, bk, wv, bv):
    b, c, h, w = x.shape
    n = h * w
    xf = x.reshape(b, c, n).astype(np.float64)
    Q = np.einsum("oc,bcn->bon", wq.astype(np.float64), xf) + bq.astype(np.float64)[None, :, None]
    K = np.einsum("oc,bcn->bon", wk.astype(np.float64), xf) + bk.astype(np.float64)[None, :, None]
    V = np.einsum("oc,bcn->bon", wv.astype(np.float64), xf) + bv.astype(np.float64)[None, :, None]
    Qn = Q / np.linalg.norm(Q, axis=1, keepdims=True)
    Kn = K / np.linalg.norm(K, axis=1, keepdims=True)
    k_sum = Kn.sum(-1) + EPS
    tailor = 1.0 / (n + np.einsum("bmn,bm->bn", Qn, k_sum))
    value_sum = V.sum(-1)
    kv = np.einsum("bmn,bcn->bmc", Kn, V)
    ms = value_sum[:, :, None] + np.einsum("bmn,bmc->bcn", Qn, kv)
    return (ms * tailor[:, None, :]).reshape(b, c, h, w).astype(np.float32)


def kernel(x, wq, bq, wk, bk, wv, bv):
    x = np.asarray(x, dtype=np.float32)
    B, Cc, H, W = x.shape
    if (any(np.any(np.asarray(b_) != 0) for b_ in (bq, bk, bv))
            or Cc != C or wq.shape != (CQK, C) or wv.shape != (C, C)
            or (H * W) % (2 * NT) != 0 or B != 4):
        return _kernel_numpy(x, wq, bq, wk, bk, wv, bv)
    N = H * W
    ncores = 8
    spb = ncores // B  # 2
    nsh = N // spb
    groups_key = tuple(
        tuple(range(b * spb, (b + 1) * spb)) for b in range(B))

    nc = _get_nc(nsh, N, ncores, groups_key)
    in_maps = make_in_maps(
        {"x": x, "wq": wq, "wk": wk, "wv": wv}, nsh)
    res = run_bass_kernel_spmd(nc, in_maps, list(range(ncores)))

    out = np.empty((B, Cc, N), np.float32)
    for core in range(ncores):
        b, hh = core // spb, core % spb
        out[b, :, hh * nsh:(hh + 1) * nsh] = \
            np.asarray(res.results[core]["out"]).astype(np.float32)
    return out.reshape(B, Cc, H, W)
